# revision 1
# baseline (speedup 1.0000x reference)
"""Multi-Head Latent Attention (MLA) Bass/Tile kernel for 8 TRN2 NeuronCores, v2c.

Sharding: 2-way data-parallel over batch x 4-way tensor-parallel over heads.
Core c = (b, g) with b = c // 4, g = c % 4 owns batch b and heads 4g..4g+3.
Each core computes a partial (S, D) output (its heads' contribution through
wo); the host sums the 4 head-group partials per batch.

Measured (bench8.py repeat-slope, 8 cores): 363-420 us/iteration across
runs (median ~385 us, +-30 us run-to-run noise) vs 438 us for the v1
baseline kernel measured the same way. TimelineSim cost model: 349 us
(v1 baseline: 414 us). Relative error vs the fp32 reference: 4.1e-3.

v2c design (all-bf16 compute; fp8 DoubleRow was tried for the latent/kv/
attention matmuls and fails the 2e-2 gate -- each piece alone contributes
2.7e-2..5.3e-2 max-relative error):
- bf16 operand storage; latent kept in SBUF (no DRAM spill).
- the latent (x @ w_kv_down) is sharded by sequence quarter across the 4
  tensor-parallel cores of each batch and AllGathered (DRAM collective):
  each core computes 1/4 of it instead of all of it redundantly.
- one fused projection pass (latent-part -> AllGather || q -> k,v).
- attention: scores in PSUM with causal -30000 masks PRE-WRITTEN into PSUM
  (matmul accumulates on top with start=False), 1024-wide batched exp,
  rowsums via ones-matmul, wo matmuls interleaved between the next chunk's
  attention heads so PE fills exp-bound stretches; bf16 output.
- q_norm*k_norm folded into kT; 4-op rope; single broadcast rinv multiply.
- one DMA instruction per logical tensor (batched APs).
"""

import os
import sys
from contextlib import ExitStack

import numpy as np

for _p in ("/opt/trn_rl_repo", os.path.expanduser("~/.axon_site/_ro/trn_rl_repo")):
    if os.path.isdir(_p) and _p not in sys.path:
        sys.path.append(_p)

import concourse.bass as bass
import concourse.bacc as bacc
import concourse.mybir as mybir
import concourse.tile as tile
from concourse.masks import make_identity

F32 = mybir.dt.float32
BF16 = mybir.dt.bfloat16
AX = mybir.AxisListType
ALU = mybir.AluOpType
ACTF = mybir.ActivationFunctionType

# Problem constants (hardcoded for nn_MultiHeadLatentAttention_74904229642374)
B, S, D, H, DK, DL, DR = 2, 2048, 2048, 16, 128, 512, 64
EPS = 1e-6
NCORES = 8
GROUPS = 4            # head-group (tensor-parallel) dimension
HG = H // GROUPS      # heads per core (4)
HD = HG * DK          # per-core head width of q/v/wo (512)
PT = 128              # partition tile
SCH = 512             # sequence chunk width
EXP_BIAS = -1.0       # exp(s*scale + EXP_BIAS); cancels in normalization
MASK_VAL = -30000.0


def build_nc(s=S, d=D, dl=DL, repeat=1, **knobs):
    nsq = s // PT          # seq tiles (16)
    nch = s // SCH         # seq chunks (4)
    nkd = d // PT          # D contraction tiles (16)
    ndl = dl // PT         # DL contraction tiles (4)
    spc = SCH // PT        # seq tiles per chunk (4)
    nno = d // SCH         # wo output col chunks (4)
    half = DR // 2
    ag = knobs.get("ag", 1) and s == S

    scale = 1.0 / float(np.sqrt(np.float32(DK)))

    nc = bacc.Bacc("TRN2", target_bir_lowering=False, debug=False,
                   num_devices=NCORES)

    xt_d = nc.dram_tensor("xt", [d, s], BF16, kind="ExternalInput")
    wq_d = nc.dram_tensor("wq", [d, HD], BF16, kind="ExternalInput")
    wkv_d = nc.dram_tensor("wkv", [d, dl], BF16, kind="ExternalInput")
    wkk_d = nc.dram_tensor("wkk", [dl, HG * DK], BF16, kind="ExternalInput")
    wv_d = nc.dram_tensor("wv", [dl, HD], BF16, kind="ExternalInput")
    wo_d = nc.dram_tensor("wo", [HD, d], BF16, kind="ExternalInput")
    cos_d = nc.dram_tensor("cos", [s, half], F32, kind="ExternalInput")
    sin_d = nc.dram_tensor("sin", [s, half], F32, kind="ExternalInput")
    qknw_d = nc.dram_tensor("qknw", [DK, 1], F32, kind="ExternalInput")
    out_d = nc.dram_tensor("out", [s, d], BF16, kind="ExternalOutput")
    if ag:
        # per-core seq-quarter slice of x.T for the latent shard
        xlat_d = nc.dram_tensor("xlat", [d, SCH], BF16, kind="ExternalInput")
        latp_d = nc.dram_tensor("latp", [dl, SCH], BF16)
        latf_d = nc.dram_tensor("latf", [GROUPS, dl, SCH], BF16)

    with tile.TileContext(nc) as tc:
      for _rep in range(repeat):
       with ExitStack() as ctx:
        const = ctx.enter_context(tc.tile_pool(name="const", bufs=1))
        big = ctx.enter_context(tc.tile_pool(name="big", bufs=1))
        stat = ctx.enter_context(tc.tile_pool(name="stat", bufs=knobs.get("stat", 6)))

        ident = const.tile([PT, PT], BF16)
        make_identity(nc, ident[:])
        ones_bf = const.tile([PT, 1], BF16)
        nc.gpsimd.memset(ones_bf[:], 1.0)
        qknw = const.tile([PT, 1], F32)
        nc.sync.dma_start(out=qknw[:], in_=qknw_d.ap())
        eps_sb = const.tile([PT, 1], F32)
        nc.gpsimd.memset(eps_sb[:], EPS)
        ebias = const.tile([PT, 1], F32)
        nc.gpsimd.memset(ebias[:], EXP_BIAS)
        cos_sb = const.tile([PT, nsq, half], F32)
        sin_sb = const.tile([PT, nsq, half], F32)
        nc.sync.dma_start(
            out=cos_sb[:],
            in_=cos_d.ap().rearrange("(t p) f -> p t f", p=PT))
        nc.sync.dma_start(
            out=sin_sb[:],
            in_=sin_d.ap().rearrange("(t p) f -> p t f", p=PT))
        # causal masks for the 4 diagonal sub-blocks: MASK_VAL where
        # key 128*dg + p > query col c (within a 512 chunk), else 0
        masks = const.tile([PT, spc, SCH], BF16)
        nc.gpsimd.memset(masks[:], 0.0)
        for dg in range(spc):
            nc.gpsimd.affine_select(
                out=masks[:, dg, :], in_=masks[:, dg, :],
                compare_op=ALU.is_ge, fill=MASK_VAL,
                base=-PT * dg, pattern=[[1, SCH]], channel_multiplier=-1)

        # persistent operand stores
        qT = big.tile([PT, HG, s], BF16)            # [dk, h, sq]
        kT = big.tile([PT, HG, s], BF16)            # [dk, h, sk]
        v_sb = big.tile([PT, nsq, HG, DK], BF16)    # [sk, tile, h, dk]
        ot = big.tile([PT, HG, s], BF16)            # [dk, h, sq]

        def rope_block(dst, src, st, tmp_pool, eng=None):
            """dst/src [128, HG, DR] APs (may alias). 4-op rope:
            A = src * [c|c]; Bt = src * [s|s]; y1 = A1 + B2; y2 = A2 - B1."""
            eng = eng or nc.vector
            src4 = src.rearrange("p h (a b) -> p h a b", a=2)
            cc = cos_sb[:, st, :].unsqueeze(1).unsqueeze(2) \
                .broadcast_to([PT, HG, 2, half])
            ss = sin_sb[:, st, :].unsqueeze(1).unsqueeze(2) \
                .broadcast_to([PT, HG, 2, half])
            a = tmp_pool.tile([PT, HG, 2, half], F32, tag="ra")
            bt = tmp_pool.tile([PT, HG, 2, half], F32, tag="rb")
            eng.tensor_mul(a[:], src4, cc)
            eng.tensor_mul(bt[:], src4, ss)
            eng.tensor_add(dst[:, :, 0:half], a[:, :, 0, :],
                           bt[:, :, 1, :])
            tsub = getattr(eng, "tensor_sub", None)
            if tsub is not None:
                tsub(dst[:, :, half:DR], a[:, :, 1, :], bt[:, :, 0, :])
            else:
                eng.tensor_tensor(dst[:, :, half:DR], a[:, :, 1, :],
                                  bt[:, :, 0, :], op=ALU.subtract)

        # ---------------- phase P: projections (latent, q, k, v) ----------------
        skip_proj = knobs.get("noproj", 0)
        with tc.tile_pool(name="wq", bufs=1) as wq_pool, \
             tc.tile_pool(name="wkv", bufs=1) as wkv_pool, \
             tc.tile_pool(name="wkk", bufs=1) as wkk_pool, \
             tc.tile_pool(name="wv", bufs=1) as wv_pool, \
             tc.tile_pool(name="lat", bufs=1) as lat_pool, \
             tc.tile_pool(name="xc", bufs=knobs.get("xb", 2)) as x_pool, \
             tc.tile_pool(name="psp", bufs=knobs.get("psp", 6), space="PSUM") as ps_proj, \
             tc.tile_pool(name="pst", bufs=knobs.get("pst", 2), space="PSUM") as pst, \
             tc.tile_pool(name="sqp", bufs=knobs.get("sqp", 2)) as sq_pool, \
             tc.tile_pool(name="qn", bufs=spc + 1) as qn_pool, \
             tc.tile_pool(name="kn", bufs=spc + 1) as kn_pool, \
             tc.tile_pool(name="rtmp", bufs=2) as rtmp:

            wq_sb = wq_pool.tile([PT, nkd, HD], BF16)
            wkv_sb = wkv_pool.tile([PT, nkd, dl], BF16)
            wkk_sb = wkk_pool.tile([PT, ndl, HG * DK], BF16)
            wv_sb = wv_pool.tile([PT, ndl, HD], BF16)
            latT = lat_pool.tile([PT, ndl, s], BF16)    # [dl-in-tile, dt, s]

            xt_r = xt_d.ap().rearrange("(k p) (c ss) -> c p k ss", p=PT, ss=SCH)

            # --- latent shard + AllGather ---
            if ag and not skip_proj:
                xl = x_pool.tile([PT, nkd, SCH], BF16, tag="xc", name="xl")
                nc.sync.dma_start(
                    out=xl[:],
                    in_=xlat_d.ap().rearrange("(k p) ss -> p k ss", p=PT))
                nc.scalar.dma_start(
                    out=wkv_sb[:],
                    in_=wkv_d.ap().rearrange("(k p) n -> p k n", p=PT))
                latp = lat_pool.tile([PT, ndl, SCH], BF16)
                for dt in range(ndl):
                    pl = ps_proj.tile([PT, SCH], F32, tag="pp")
                    for k in range(nkd):
                        nc.tensor.matmul(
                            pl[:], wkv_sb[:, k, dt * PT:(dt + 1) * PT],
                            xl[:, k, :],
                            start=(k == 0), stop=(k == nkd - 1))
                    nc.scalar.copy(latp[:, dt, :], pl[:])
                nc.scalar.dma_start(
                    out=latp_d.ap().rearrange("(dt p) ss -> p dt ss", p=PT),
                    in_=latp[:])
                nc.gpsimd.collective_compute(
                    "AllGather", ALU.bypass,
                    replica_groups=[[0, 1, 2, 3], [4, 5, 6, 7]],
                    ins=[latp_d.ap()], outs=[latf_d.ap()])


            # --- q projection, all chunks (overlaps the AllGather) ---
            qns = []
            for c in range(nch if not skip_proj else 0):
                xc = x_pool.tile([PT, nkd, SCH], BF16, tag="xc")
                nc.sync.dma_start(out=xc[:], in_=xt_r[c])
                if c == 0:
                    nc.sync.dma_start(
                        out=wq_sb[:],
                        in_=wq_d.ap().rearrange("(k p) n -> p k n", p=PT))
                    if not ag:
                        nc.sync.dma_start(
                            out=wkv_sb[:],
                            in_=wkv_d.ap().rearrange("(k p) n -> p k n", p=PT))
                    nc.sync.dma_start(
                        out=wkk_sb[:],
                        in_=wkk_d.ap().rearrange("(k p) n -> p k n", p=PT))
                    nc.sync.dma_start(
                        out=wv_sb[:],
                        in_=wv_d.ap().rearrange("(k p) n -> p k n", p=PT))

                if not ag:
                    for dt in range(ndl):
                        pl = ps_proj.tile([PT, SCH], F32, tag="pp")
                        for k in range(nkd):
                            nc.tensor.matmul(
                                pl[:], wkv_sb[:, k, dt * PT:(dt + 1) * PT],
                                xc[:, k, :],
                                start=(k == 0), stop=(k == nkd - 1))
                        nc.scalar.copy(latT[:, dt, c * SCH:(c + 1) * SCH], pl[:])

                for t in range(spc):
                    st = c * spc + t
                    pq = ps_proj.tile([PT, HD], F32, tag="pp")
                    for k in range(nkd):
                        nc.tensor.matmul(
                            pq[:], xc[:, k, t * PT:(t + 1) * PT], wq_sb[:, k, :],
                            start=(k == 0), stop=(k == nkd - 1))
                    sq = sq_pool.tile([PT, HD], F32, tag="sq")
                    nc.scalar.activation(sq[:], pq[:], ACTF.Square)
                    red = stat.tile([PT, HG], F32, tag="redq")
                    nc.vector.tensor_reduce(
                        red[:], sq[:].rearrange("p (h w) -> p h w", h=HG),
                        axis=AX.X, op=ALU.add)
                    rinv = stat.tile([PT, HG], F32, tag="rinvq")
                    nc.scalar.activation(rinv[:], red[:], ACTF.Sqrt,
                                         scale=1.0 / DK, bias=eps_sb[:, 0:1])
                    nc.vector.reciprocal(rinv[:], rinv[:])
                    qn = qn_pool.tile([PT, HG, DK], BF16, tag="qn")
                    nc.vector.tensor_mul(
                        qn[:], pq[:].rearrange("p (h e) -> p h e", h=HG),
                        rinv[:].unsqueeze(2).broadcast_to([PT, HG, DK]))
                    rope_block(qn[:, :, 0:DR], qn[:, :, 0:DR], st, rtmp)
                    qns.append((st, qn))
                    if st >= 2:
                        # transpose the tile from 2 tiles ago (norm chain hidden)
                        sp, qp = qns[st - 2]
                        tp = pst.tile([PT, HG * PT], BF16, tag="tp")
                        for h in range(HG):
                            nc.tensor.transpose(tp[:, h * PT:(h + 1) * PT],
                                                qp[:, h, :], ident[:])
                        nc.vector.tensor_copy(
                            qT[:, :, sp * PT:(sp + 1) * PT],
                            tp[:].rearrange("p (h w) -> p h w", h=HG))

            for sp, qp in qns[nsq - 2:] if not skip_proj else []:
                tp = pst.tile([PT, HG * PT], BF16, tag="tp")
                for h in range(HG):
                    nc.tensor.transpose(tp[:, h * PT:(h + 1) * PT],
                                        qp[:, h, :], ident[:])
                nc.vector.tensor_copy(
                    qT[:, :, sp * PT:(sp + 1) * PT],
                    tp[:].rearrange("p (h w) -> p h w", h=HG))

            # --- k, v from the gathered latent ---
            if ag and not skip_proj:
                for cg in range(GROUPS):
                    nc.sync.dma_start(
                        out=latT[:, :, cg * SCH:(cg + 1) * SCH],
                        in_=latf_d.ap()[cg].rearrange("(dt p) j -> p dt j",
                                                      p=PT))
            if skip_proj:
                # timing-only mode: fill operand stores so attention can run
                nc.gpsimd.memset(qT[:], 0.01)
                nc.gpsimd.memset(kT[:], 0.01)
                nc.gpsimd.memset(v_sb[:], 0.01)
            kns = []
            for st in range(nsq if not skip_proj else 0):
                pkk = ps_proj.tile([PT, HG * DK], F32, tag="pp")
                pv = ps_proj.tile([PT, HD], F32, tag="pp")
                for dt in range(ndl):
                    lt = latT[:, dt, st * PT:(st + 1) * PT]
                    nc.tensor.matmul(pkk[:], lt, wkk_sb[:, dt, :],
                                     start=(dt == 0), stop=(dt == ndl - 1))
                    nc.tensor.matmul(pv[:], lt, wv_sb[:, dt, :],
                                     start=(dt == 0), stop=(dt == ndl - 1))
                pknv = pkk[:, HG * DR:HG * DK].rearrange("p (h e) -> p h e", h=HG)
                kraw = sq_pool.tile([PT, HG, DR], F32, tag="kraw")
                nc.scalar.copy(
                    kraw[:].rearrange("p h e -> p (h e)"), pkk[:, 0:HG * DR])
                sqk = sq_pool.tile([PT, HG * DK], F32, tag="sq")
                nc.scalar.activation(sqk[:], pkk[:], ACTF.Square)
                r1 = stat.tile([PT, HG], F32, tag="r1")
                nc.vector.tensor_reduce(
                    r1[:],
                    sqk[:].rearrange("p (g h w) -> p h g w", g=2, h=HG),
                    axis=AX.XY, op=ALU.add)
                rinv = stat.tile([PT, HG], F32, tag="rinvk")
                nc.scalar.activation(rinv[:], r1[:], ACTF.Sqrt,
                                     scale=1.0 / DK, bias=eps_sb[:, 0:1])
                nc.vector.reciprocal(rinv[:], rinv[:])
                kn = kn_pool.tile([PT, HG, DK], BF16, tag="kn")
                rope_block(kn[:, :, 0:DR], kraw[:], st, rtmp,
                           eng=nc.gpsimd if knobs.get("kpool", 1) else nc.vector)
                nc.scalar.copy(kn[:, :, DR:DK], pknv)
                nc.vector.tensor_mul(
                    kn[:], kn[:],
                    rinv[:].unsqueeze(2).broadcast_to([PT, HG, DK]))
                nc.scalar.copy(
                    v_sb[:, st, :, :].rearrange("p h e -> p (h e)"), pv[:])
                kns.append((st, kn))
                if st >= 2:
                    sp, kp = kns[st - 2]
                    tp = pst.tile([PT, HG * PT], BF16, tag="tp")
                    for h in range(HG):
                        nc.tensor.transpose(tp[:, h * PT:(h + 1) * PT],
                                            kp[:, h, :], ident[:])
                    # fold q_norm_w * k_norm_w (per-dk) into kT
                    nc.vector.tensor_scalar(
                        kT[:, :, sp * PT:(sp + 1) * PT],
                        tp[:].rearrange("p (h w) -> p h w", h=HG),
                        qknw[:, 0:1], None, op0=ALU.mult)
            for sp, kp in kns[nsq - 2:] if not skip_proj else []:
                tp = pst.tile([PT, HG * PT], BF16, tag="tp")
                for h in range(HG):
                    nc.tensor.transpose(tp[:, h * PT:(h + 1) * PT],
                                        kp[:, h, :], ident[:])
                nc.vector.tensor_scalar(
                    kT[:, :, sp * PT:(sp + 1) * PT],
                    tp[:].rearrange("p (h w) -> p h w", h=HG),
                    qknw[:, 0:1], None, op0=ALU.mult)

        wo_sb_holder = [None]
        # ---------------- phase A: attention + wo, per chunk ----------------
        if knobs.get("noattn"):
            # keep the proj work alive: dump operand stores to out
            nc.sync.dma_start(out=out_d.ap()[0:PT, :],
                              in_=qT[:].rearrange("p h s -> p (h s)")[:, 0:d])
            nc.sync.dma_start(out=out_d.ap()[PT:2 * PT, :],
                              in_=kT[:].rearrange("p h s -> p (h s)")[:, 0:d])
            nc.sync.dma_start(out=out_d.ap()[2 * PT:3 * PT, :],
                              in_=v_sb[:].rearrange("p t h e -> p (t h e)")[:, 0:d])
            nc.sync.dma_start(out=out_d.ap()[3 * PT:4 * PT, :],
                              in_=latT[:].rearrange("p a s -> p (a s)")[:, 0:d])
            continue
        wosep = knobs.get("wosep", 0)
        wo_stack = ExitStack()
        with tc.tile_pool(name="wo", bufs=1) as wo_pool, \
             tc.tile_pool(name="pb", bufs=knobs.get("pb", 3)) as pb_pool, \
             tc.tile_pool(name="bcast", bufs=2) as bc_pool, \
             tc.tile_pool(name="outst", bufs=knobs.get("outst", 2)) as out_pool, \
             tc.tile_pool(name="pssc", bufs=knobs.get("pssc", 3 if wosep else 2), space="PSUM") as pssc, \
             tc.tile_pool(name="psot", bufs=knobs.get("psot", 1), space="PSUM") as psot, \
             tc.tile_pool(name="psrs", bufs=knobs.get("psrs", 1), space="PSUM") as psrs:
            if wosep:
                pswo = None
            else:
                pswo = wo_stack.enter_context(
                    tc.tile_pool(name="pswo", bufs=knobs.get("pswo", 2),
                                 space="PSUM"))

            wo_sb = big.tile([PT, HG, d], BF16, name="wo_sb")
            nc.sync.dma_start(
                out=wo_sb[:],
                in_=wo_d.ap().rearrange("(h p) n -> p h n", p=PT))
            wo_sb_holder[0] = wo_sb

            pswo_ref = [pswo]

            def wo_emit(st, n):
                pw = pswo_ref[0].tile([PT, SCH], F32, tag="pswo")
                for h in range(HG):
                    nc.tensor.matmul(
                        pw[:],
                        ot[:, h, st * PT:(st + 1) * PT],
                        wo_sb[:, h, n * SCH:(n + 1) * SCH],
                        start=(h == 0), stop=(h == HG - 1))
                ob, filled = ob_state[0]
                nc.vector.tensor_copy(ob[:, n * SCH:(n + 1) * SCH], pw[:])
                filled.add(n)
                if len(filled) == nno:
                    nc.sync.dma_start(
                        out=out_d.ap()[st * PT:(st + 1) * PT, :], in_=ob[:])

            wo_queue = []
            ob_state = [None]
            wodg = knobs.get("wodg", 0)

            def wo_drain(k):
                for _ in range(k):
                    if not wo_queue:
                        return
                    st, n = wo_queue.pop(0)
                    if n == 0:
                        ob = out_pool.tile([PT, d], BF16, tag="outst",
                                           name="ob")
                        ob_state[0] = (ob, set())
                    wo_emit(st, n)

            for cj in range(nch):
                nsk = spc * cj + spc
                for h in range(HG):
                    po = psot.tile([PT, SCH], F32, tag="psot")
                    prs = psrs.tile([1, SCH], F32, tag="psrs")
                    ngrp = nsk // 2

                    def emit_av(g, pb, c0s):
                        for j, i in enumerate((2 * g, 2 * g + 1)):
                            c0 = c0s[j]
                            nc.tensor.matmul(
                                po[:, c0:SCH],
                                v_sb[:, i, h, :],
                                pb[:, j, c0:SCH],
                                start=(i == 0), stop=(i == nsk - 1))
                            nc.tensor.matmul(
                                prs[:, c0:SCH],
                                ones_bf[:],
                                pb[:, j, c0:SCH],
                                start=(i == 0), stop=(i == nsk - 1))

                    prev = None
                    for g in range(ngrp):
                        blocks = (2 * g, 2 * g + 1)
                        psc = pssc.tile([PT, 2, SCH], F32, tag="pssc")
                        pb = pb_pool.tile([PT, 2, SCH], BF16, tag="pb")
                        c0s = []
                        for j, i in enumerate(blocks):
                            # pre-write causal mask into PSUM; scores matmul
                            # accumulates on top (start=False) so the mask
                            # write stays off the scores->exp->AV chain
                            dg = i - spc * cj
                            c0 = 0 if dg < 0 else PT * dg
                            c0s.append(c0)
                            if dg >= 0:
                                nc.vector.tensor_copy(psc[:, j, :],
                                                      masks[:, dg, :])
                            nc.tensor.matmul(
                                psc[:, j, c0:SCH],
                                kT[:, h, i * PT:(i + 1) * PT],
                                qT[:, h, cj * SCH + c0:(cj + 1) * SCH],
                                start=(dg < 0), stop=True,
                                skip_group_check=(dg >= 0))
                        pscf = psc[:].rearrange("p a b -> p (a b)")
                        pbf = pb[:].rearrange("p a b -> p (a b)")
                        nc.scalar.activation(pbf[:], pscf[:], ACTF.Exp,
                                             scale=scale, bias=ebias[:, 0:1])
                        # software pipeline: AV of the PREVIOUS group goes
                        # behind this group's scores so the in-order PE queue
                        # never head-of-line blocks on this group's exp
                        if prev is not None:
                            emit_av(*prev)
                        if wodg and not wosep:
                            wo_drain(wodg)
                        prev = (g, pb, c0s)
                    emit_av(*prev)
                    rs = bc_pool.tile([1, SCH], F32, tag="rs")
                    nc.vector.tensor_copy(rs[:], prs[:])
                    nc.vector.reciprocal(rs[:], rs[:])
                    bc = bc_pool.tile([PT, SCH], F32, tag="bc")
                    nc.gpsimd.partition_broadcast(bc[:], rs[:], channels=PT)
                    nc.vector.tensor_mul(
                        ot[:, h, cj * SCH:(cj + 1) * SCH], po[:], bc[:])
                    if not wosep:
                        wo_drain(knobs.get("wod", 4))

                # queue this chunk's wo units; they are emitted interleaved
                # between the NEXT chunk's attention heads so PE fills
                # ACT(exp)-bound stretches
                for t in range(spc):
                    st = cj * spc + t
                    for n in range(nno):
                        wo_queue.append((st, n))
                if cj == nch - 1 and not wosep:
                    wo_drain(len(wo_queue))

            wo_stack.close()
        if knobs.get("wosep", 0) and not knobs.get("noattn"):
            with tc.tile_pool(name="wo2", bufs=1) as wo2_pool, \
                 tc.tile_pool(name="outst2", bufs=2) as out2_pool, \
                 tc.tile_pool(name="pswo2", bufs=knobs.get("pswo", 3),
                              space="PSUM") as pswo2:
                for st in range(nsq):
                    ob = out2_pool.tile([PT, d], BF16, tag="ob2")
                    for n in range(nno):
                        pw = pswo2.tile([PT, SCH], F32, tag="pw2")
                        for h in range(HG):
                            nc.tensor.matmul(
                                pw[:],
                                ot[:, h, st * PT:(st + 1) * PT],
                                wo_sb_holder[0][:, h, n * SCH:(n + 1) * SCH],
                                start=(h == 0), stop=(h == HG - 1))
                        nc.vector.tensor_copy(ob[:, n * SCH:(n + 1) * SCH],
                                              pw[:])
                    nc.sync.dma_start(
                        out=out_d.ap()[st * PT:(st + 1) * PT, :], in_=ob[:])

    nc.compile()
    return nc


def rope_tables(s):
    quarter = DR // 4
    freq = (1.0 / 10000.0) ** np.linspace(0.0, 1.0, quarter, dtype=np.float32)
    freq = np.concatenate([freq, np.zeros((quarter,), np.float32)])
    theta = np.arange(s, dtype=np.float32)[:, None] * freq[None, :]
    return np.cos(theta).astype(np.float32), np.sin(theta).astype(np.float32)


def make_in_maps(x, wq, w_kv_down, w_k_rope, w_k_nope, wv, wo,
                 q_norm_w, k_norm_w):
    import ml_dtypes
    bf = ml_dtypes.bfloat16
    s = x.shape[1]
    cos, sin = rope_tables(s)
    ca = np.ascontiguousarray
    qknw = (np.asarray(q_norm_w) * np.asarray(k_norm_w)).astype(np.float32)
    in_maps = []
    for c in range(NCORES):
        b, g = divmod(c, GROUPS)
        xtb = ca(x[b].T).astype(bf)
        wkk = np.concatenate(
            [w_k_rope[:, g * HG * DR:(g + 1) * HG * DR],
             w_k_nope[:, g * HG * (DK - DR):(g + 1) * HG * (DK - DR)]],
            axis=1)
        in_maps.append({
            "xt": xtb,
            "xlat": ca(xtb[:, g * SCH:(g + 1) * SCH]),
            "wq": ca(wq[:, g * HD:(g + 1) * HD]).astype(bf),
            "wkv": ca(w_kv_down).astype(bf),
            "wkk": ca(wkk).astype(bf),
            "wv": ca(wv[:, g * HD:(g + 1) * HD]).astype(bf),
            "wo": ca(wo[g * HD:(g + 1) * HD, :]).astype(bf),
            "cos": cos, "sin": sin,
            "qknw": ca(qknw.reshape(DK, 1)),
        })
    return in_maps


_NC_CACHE = {}


def run(inputs, trace=False, **kwargs):
    from concourse.bass_utils import run_bass_kernel_spmd
    if "nc" not in _NC_CACHE:
        _NC_CACHE["nc"] = build_nc()
    nc = _NC_CACHE["nc"]
    in_maps = make_in_maps(**inputs)
    res = run_bass_kernel_spmd(nc, in_maps, core_ids=list(range(NCORES)),
                               trace=trace, **kwargs)
    outs = [r["out"] for r in res.results]
    full = np.zeros((B, S, D), np.float32)
    for b in range(B):
        for g in range(GROUPS):
            full[b] += np.asarray(outs[b * GROUPS + g], np.float32)
    return full, res


def kernel(**inputs):
    out, _ = run(inputs)
    return out

